# revision 14
# baseline (speedup 1.0000x reference)
"""Trainium2 Bass kernel for nn_AttentionBlock (B=4, S=2048, E=1024, H=16, D=64).

Sharding: 8 cores, core c -> batch c//2, query rows (c%2)*1024 ..+1024.
Each core computes all 16 heads for its 1024 queries against the batch's full
K/V, then the output projection locally (no collectives needed).

Per-core program (SPMD, identical program, per-core input data):
  scoresT[k,q] = KT_h_chunk.T @ QT_h  on PE (f32r), exp on ACT out of PSUM
  (no max-subtraction: scores ~ N(0,1)), AV matmul accumulates V_ext.T @ pT
  where V_ext has a ones column so row 64 of the output is the softmax
  denominator; normalize via DVE reciprocal + gpsimd partition_broadcast +
  DVE multiply, landing attnT[e,q] ready as projection stationary operand.
"""

from contextlib import ExitStack

import numpy as np

import concourse.bass as bass  # noqa: F401
import concourse.mybir as mybir
import concourse.tile as tile
from concourse import bacc
from concourse.bass_utils import run_bass_kernel_spmd
from concourse import dve_ops as _dvo
from concourse.dve_spec import Spec, Src0, C0, One, sq, lower
from concourse.dve_uop import DveOpSpec
from concourse.dve_table_gen import dve_ver_for

F32 = mybir.dt.float32
F32R = mybir.dt.float32r
BF16 = mybir.dt.bfloat16
FP16 = mybir.dt.float16
EXP = mybir.ActivationFunctionType.Exp

def _register_exp_ops():
    """exp(x) ~= (1 + x*scale/4096)^4096 as two chained DVE squaring ops.

    Offloads part of the softmax exp from the (bottleneck) scalar engine to
    the vector engine. Approximation bias exp(x)*x^2/8192 < 0.4% at |x|<5.5.
    """
    names = ("EXP_SQA_ANT", "EXP_SQB_ANT")
    if names[0] in _dvo.CUSTOM_DVE_SPECS:
        by_name = {op.name: op for op in _dvo.OPS}
        return by_name[names[0]], by_name[names[1]]

    def _ref_a(in0, in1, c0, c1, c2):
        r = (np.float32(1.0) + in0.astype(np.float32) * np.float32(c0)).astype(np.float32)
        for _ in range(6):
            r = (r * r).astype(np.float32)
        return r

    def _ref_b(in0, in1, c0, c1, c2):
        r = in0.astype(np.float32)
        for _ in range(6):
            r = (r * r).astype(np.float32)
        return r

    body_a = One + Src0 * C0
    for _ in range(6):
        body_a = sq(body_a)
    body_b = Src0
    for _ in range(6):
        body_b = sq(body_b)

    out = []
    ver = dve_ver_for("TRN2")
    for name, body, ref in ((names[0], body_a, _ref_a), (names[1], body_b, _ref_b)):
        spec = Spec(body=body, reference=ref)
        row = max(_dvo._SUB_OPCODE_FOR_NAME.values()) + 1
        _dvo._SUB_OPCODE_FOR_NAME[name] = row
        tmpspec = DveOpSpec(name=name, opcode=row, uops=lower(spec, ver=ver),
                            rd1_en=False)
        op = _dvo.DveOp(name, spec, subdim=False, uops_sha={ver: tmpspec.sha(ver)})
        _dvo.OPS.append(op)
        _dvo.CUSTOM_DVE_SPECS[name] = spec
        out.append(op)
    return out


EXP_A, EXP_B = _register_exp_ops()

B, S, E, H, D = 4, 2048, 1024, 16, 64
NQ = S // 2
N_CORES = 8

_CACHE: dict = {}


def build_attn_program(S=S, NQ=NQ, E=E, H=H, D=D, reps=1, dt_mode="mixed", pipe=True,
                       dve_exp_mod=4):
    assert E == H * D and D == 64
    KT = S // 128
    ST = NQ // 128
    EC = E // 128
    HP = H // 2
    scale = float(D) ** -0.5

    wdt = BF16 if dt_mode == "bf16" else F32R
    nc = bacc.Bacc("TRN2", target_bir_lowering=False, debug=False, num_devices=8)
    qT = nc.dram_tensor("qT", [E, NQ], wdt, kind="ExternalInput").ap()
    kTd = nc.dram_tensor("kT", [E, S], wdt, kind="ExternalInput").ap()
    vd = nc.dram_tensor("vext", [S, H * 65], BF16, kind="ExternalInput").ap()
    wTd = nc.dram_tensor("wT", [E, E], wdt, kind="ExternalInput").ap()
    o = nc.dram_tensor("o", [NQ, E], F32, kind="ExternalOutput").ap()

    def mm(out, lhsT, rhs, start, stop):
        # matmul output must fit one PSUM bank: free dim <= 512
        step = 512
        n = rhs.shape[-1]
        for c0 in range(0, n, step):
            c1 = min(c0 + step, n)
            nc.tensor.matmul(
                out[:, c0:c1], lhsT, rhs[:, c0:c1], start=start, stop=stop,
            )

    def body(tc, ctx):
        vext_pool = ctx.enter_context(tc.tile_pool(name="vext", bufs=KT))
        wt_pool = ctx.enter_context(tc.tile_pool(name="wt", bufs=EC))
        qt_pool = ctx.enter_context(tc.tile_pool(name="qt", bufs=2))
        kt_pool = ctx.enter_context(tc.tile_pool(name="kt", bufs=2))
        pt_pool = ctx.enter_context(tc.tile_pool(name="pt", bufs=4))
        dt_pool = ctx.enter_context(tc.tile_pool(name="dt", bufs=2))
        at_pool = ctx.enter_context(tc.tile_pool(name="at", bufs=EC))
        rc_pool = ctx.enter_context(tc.tile_pool(name="rc", bufs=2))
        rb_pool = ctx.enter_context(tc.tile_pool(name="rb", bufs=2))
        ones_pool = ctx.enter_context(tc.tile_pool(name="ones", bufs=1))
        ot_pool = ctx.enter_context(tc.tile_pool(name="ot", bufs=2))
        st_psum = ctx.enter_context(tc.tile_pool(name="stp", bufs=2, space="PSUM"))
        av_psum = ctx.enter_context(tc.tile_pool(name="avp", bufs=2, space="PSUM"))

        # ---- prolog: V_ext chunks (host-built: per-head 64 v cols + ones) ----
        vext = []
        for i in range(KT):
            ve = vext_pool.tile([128, H * 65], BF16, name=f"vext{i}", tag="vext")
            nc.sync.dma_start(ve[:], vd[128 * i : 128 * (i + 1), :])
            vext.append(ve)

        wts = []
        for e in range(EC):
            wt = wt_pool.tile([128, E], wdt, name=f"wt{e}", tag="wtt")
            nc.sync.dma_start(wt[:], wTd[128 * e : 128 * (e + 1), :])
            wts.append(wt)

        # ones [32, 64] stationary for the K=32 recip-broadcast outer product
        ones32 = ones_pool.tile([32, 64], FP16, name="ones32", tag="ones32")
        nc.vector.memset(ones32[:], 1.0)

        # ---- attention ----
        attnT = []
        for j in range(HP):
            qtp = qt_pool.tile([128, NQ], wdt, name=f"qtp{j}", tag="qtp")
            nc.sync.dma_start(qtp[:], qT[128 * j : 128 * (j + 1), :])
            ktp = kt_pool.tile([128, S], wdt, name=f"ktp{j}", tag="ktp")
            nc.sync.dma_start(ktp[:], kTd[128 * j : 128 * (j + 1), :])
            at = at_pool.tile([128, NQ], wdt, name=f"at{j}", tag="at")
            attnT.append(at)
            for hh in range(2):
                h = 2 * j + hh
                r0, r1 = 64 * hh, 64 * hh + 64
                av = av_psum.tile([65, NQ], F32, name=f"av{h}", tag="av")
                pending = None
                for i in range(KT):
                    st = st_psum.tile([128, NQ], F32, name=f"st{h}_{i}", tag="st")
                    mm(st, ktp[r0:r1, 128 * i : 128 * (i + 1)], qtp[r0:r1, :],
                       start=True, stop=True)
                    pt = pt_pool.tile([128, NQ], BF16, name=f"pt{h}_{i}", tag="pt")
                    if dve_exp_mod and i % dve_exp_mod == 1:
                        td = dt_pool.tile([128, NQ], F32, name=f"td{h}_{i}", tag="td")
                        nc.vector._custom_dve(EXP_A, out=td[:], in0=st[:],
                                              s0=scale / 4096.0)
                        nc.vector._custom_dve(EXP_B, out=pt[:], in0=td[:])
                    else:
                        nc.scalar.activation(pt[:], st[:], EXP, scale=scale)
                    if not pipe:
                        mm(av, vext[i][:, 65 * h : 65 * h + 65], pt[:],
                           start=(i == 0), stop=(i == KT - 1))
                        continue
                    # pipelined: issue av(i-1) after scores(i) so PE never
                    # waits on exp(i)
                    if pending is not None:
                        mm(av, pending[0], pending[1],
                           start=(pending[2] == 0), stop=False)
                    pending = (vext[i][:, 65 * h : 65 * h + 65], pt, i)
                if pipe:
                    ve_ap, pt_ap, i = pending
                    mm(av, ve_ap, pt_ap, start=(i == 0), stop=True)
                rc = rc_pool.tile([32, NQ], FP16, name=f"rc{h}", tag="rc")
                nc.vector.memset(rc[:], 0.0)
                with nc.allow_low_precision(reason="fp16 softmax-recip broadcast"):
                    nc.vector.reciprocal(rc[0:1, :], av[64:65, :])
                bc = av_psum.tile([64, NQ], F32, name=f"bc{h}", tag="av")
                for c0 in range(0, NQ, 512):
                    c1 = min(c0 + 512, NQ)
                    nc.tensor.matmul(bc[:, c0:c1], ones32[:],
                                     rc[:, c0:c1], start=True, stop=True)
                rb = rb_pool.tile([64, NQ], F32, name=f"rb{h}", tag="rb")
                nc.vector.tensor_copy(rb[:], bc[:])
                nc.vector.tensor_mul(at[r0:r1, :], av[0:64, :], rb[:])

        # ---- projection ----
        for t in range(ST):
            pp = st_psum.tile([128, E], F32, name=f"pp{t}", tag="st")
            for e in range(EC):
                mm(pp, attnT[e][:, 128 * t : 128 * (t + 1)], wts[e][:],
                   start=(e == 0), stop=(e == EC - 1))
            ot = ot_pool.tile([128, E], F32, name=f"ot{t}", tag="ot")
            nc.vector.tensor_copy(ot[:], pp[:])
            nc.sync.dma_start(o[128 * t : 128 * (t + 1), :], ot[:])

    with ExitStack() as ctx:
        tc = ctx.enter_context(tile.TileContext(nc))
        if reps == 1:
            body(tc, ctx)
        else:
            with tc.For_i(0, reps, 1):
                body(tc, ctx)

    nc.compile()
    return nc


DT_MODE = "bf16"
PIPE = True
DVE_EXP_MOD = 0


def _get_program(reps=1):
    key = ("prog", reps, DT_MODE, PIPE, DVE_EXP_MOD)
    if key not in _CACHE:
        _CACHE[key] = build_attn_program(reps=reps, dt_mode=DT_MODE, pipe=PIPE,
                                         dve_exp_mod=DVE_EXP_MOD)
    return _CACHE[key]


def make_in_maps(q, k, v, w_out):
    import ml_dtypes
    qT = np.ascontiguousarray(np.transpose(q.reshape(B, 2, NQ, E), (0, 1, 3, 2)))
    kT = np.ascontiguousarray(np.transpose(k, (0, 2, 1)))
    wT = np.ascontiguousarray(w_out.T)
    if DT_MODE == "bf16":
        qT = qT.astype(ml_dtypes.bfloat16)
        kT = kT.astype(ml_dtypes.bfloat16)
        wT = wT.astype(ml_dtypes.bfloat16)
    vext = np.empty((B, S, H, 65), dtype=ml_dtypes.bfloat16)
    vext[:, :, :, 0:64] = v.reshape(B, S, H, 64)
    vext[:, :, :, 64] = 1.0
    vext = vext.reshape(B, S, H * 65)
    in_maps = []
    for c in range(N_CORES):
        b, half = divmod(c, 2)
        in_maps.append({"qT": qT[b, half], "kT": kT[b], "vext": vext[b], "wT": wT})
    return in_maps


def kernel(q, k, v, w_out, b_out, _reps=1):
    q = np.asarray(q, np.float32)
    k = np.asarray(k, np.float32)
    v = np.asarray(v, np.float32)
    w_out = np.asarray(w_out, np.float32)
    b_out = np.asarray(b_out, np.float32)

    nc = _get_program(_reps)
    res = run_bass_kernel_spmd(nc, make_in_maps(q, k, v, w_out),
                               core_ids=list(range(N_CORES)))
    out = np.empty((B, S, E), np.float32)
    for c in range(N_CORES):
        b, half = divmod(c, 2)
        out[b, half * NQ : (half + 1) * NQ] = res.results[c]["o"]
    out += b_out[None, None, :]
    return out


# revision 15
# speedup vs baseline: 1.0081x; 1.0081x over previous
"""Trainium2 Bass kernel for nn_AttentionBlock (B=4, S=2048, E=1024, H=16, D=64).

Sharding: 8 cores, core c -> batch c//2, query rows (c%2)*1024 ..+1024.
Each core computes all 16 heads for its 1024 queries against the batch's full
K/V, then the output projection locally (no collectives needed).

Per-core program (SPMD, identical program, per-core input data):
  scoresT[k,q] = KT_h_chunk.T @ QT_h  on PE (f32r), exp on ACT out of PSUM
  (no max-subtraction: scores ~ N(0,1)), AV matmul accumulates V_ext.T @ pT
  where V_ext has a ones column so row 64 of the output is the softmax
  denominator; normalize via DVE reciprocal + gpsimd partition_broadcast +
  DVE multiply, landing attnT[e,q] ready as projection stationary operand.
"""

from contextlib import ExitStack

import numpy as np

import concourse.bass as bass  # noqa: F401
import concourse.mybir as mybir
import concourse.tile as tile
from concourse import bacc
from concourse.bass_utils import run_bass_kernel_spmd
from concourse import dve_ops as _dvo
from concourse.dve_spec import Spec, Src0, C0, One, sq, lower
from concourse.dve_uop import DveOpSpec
from concourse.dve_table_gen import dve_ver_for

F32 = mybir.dt.float32
F32R = mybir.dt.float32r
BF16 = mybir.dt.bfloat16
FP16 = mybir.dt.float16
EXP = mybir.ActivationFunctionType.Exp

def _register_exp_ops():
    """exp(x) ~= (1 + x*scale/4096)^4096 as two chained DVE squaring ops.

    Offloads part of the softmax exp from the (bottleneck) scalar engine to
    the vector engine. Approximation bias exp(x)*x^2/8192 < 0.4% at |x|<5.5.
    """
    names = ("EXP_SQA_ANT", "EXP_SQB_ANT")
    if names[0] in _dvo.CUSTOM_DVE_SPECS:
        by_name = {op.name: op for op in _dvo.OPS}
        return by_name[names[0]], by_name[names[1]]

    def _ref_a(in0, in1, c0, c1, c2):
        r = (np.float32(1.0) + in0.astype(np.float32) * np.float32(c0)).astype(np.float32)
        for _ in range(6):
            r = (r * r).astype(np.float32)
        return r

    def _ref_b(in0, in1, c0, c1, c2):
        r = in0.astype(np.float32)
        for _ in range(6):
            r = (r * r).astype(np.float32)
        return r

    body_a = One + Src0 * C0
    for _ in range(6):
        body_a = sq(body_a)
    body_b = Src0
    for _ in range(6):
        body_b = sq(body_b)

    out = []
    ver = dve_ver_for("TRN2")
    for name, body, ref in ((names[0], body_a, _ref_a), (names[1], body_b, _ref_b)):
        spec = Spec(body=body, reference=ref)
        row = max(_dvo._SUB_OPCODE_FOR_NAME.values()) + 1
        _dvo._SUB_OPCODE_FOR_NAME[name] = row
        tmpspec = DveOpSpec(name=name, opcode=row, uops=lower(spec, ver=ver),
                            rd1_en=False)
        op = _dvo.DveOp(name, spec, subdim=False, uops_sha={ver: tmpspec.sha(ver)})
        _dvo.OPS.append(op)
        _dvo.CUSTOM_DVE_SPECS[name] = spec
        out.append(op)
    return out


B, S, E, H, D = 4, 2048, 1024, 16, 64
NQ = S // 2
N_CORES = 8

_CACHE: dict = {}


def build_attn_program(S=S, NQ=NQ, E=E, H=H, D=D, reps=1, dt_mode="mixed", pipe=True,
                       dve_exp_mod=0):
    exp_ops = _register_exp_ops() if dve_exp_mod else None
    assert E == H * D and D == 64
    KT = S // 128
    ST = NQ // 128
    EC = E // 128
    HP = H // 2
    scale = float(D) ** -0.5

    wdt = BF16 if dt_mode == "bf16" else F32R
    nc = bacc.Bacc("TRN2", target_bir_lowering=False, debug=False, num_devices=8)
    qT = nc.dram_tensor("qT", [E, NQ], wdt, kind="ExternalInput").ap()
    kTd = nc.dram_tensor("kT", [E, S], wdt, kind="ExternalInput").ap()
    vd = nc.dram_tensor("vext", [S, H * 65], BF16, kind="ExternalInput").ap()
    wTd = nc.dram_tensor("wT", [E, E], wdt, kind="ExternalInput").ap()
    o = nc.dram_tensor("o", [NQ, E], F32, kind="ExternalOutput").ap()

    def mm(out, lhsT, rhs, start, stop):
        # matmul output must fit one PSUM bank: free dim <= 512
        step = 512
        n = rhs.shape[-1]
        for c0 in range(0, n, step):
            c1 = min(c0 + step, n)
            nc.tensor.matmul(
                out[:, c0:c1], lhsT, rhs[:, c0:c1], start=start, stop=stop,
            )

    def body(tc, ctx):
        vext_pool = ctx.enter_context(tc.tile_pool(name="vext", bufs=KT))
        wt_pool = ctx.enter_context(tc.tile_pool(name="wt", bufs=EC))
        qt_pool = ctx.enter_context(tc.tile_pool(name="qt", bufs=2))
        kt_pool = ctx.enter_context(tc.tile_pool(name="kt", bufs=2))
        pt_pool = ctx.enter_context(tc.tile_pool(name="pt", bufs=4))
        dt_pool = ctx.enter_context(tc.tile_pool(name="dt", bufs=2))
        at_pool = ctx.enter_context(tc.tile_pool(name="at", bufs=EC))
        rc_pool = ctx.enter_context(tc.tile_pool(name="rc", bufs=2))
        rb_pool = ctx.enter_context(tc.tile_pool(name="rb", bufs=2))
        ones_pool = ctx.enter_context(tc.tile_pool(name="ones", bufs=1))
        ot_pool = ctx.enter_context(tc.tile_pool(name="ot", bufs=2))
        st_psum = ctx.enter_context(tc.tile_pool(name="stp", bufs=2, space="PSUM"))
        av_psum = ctx.enter_context(tc.tile_pool(name="avp", bufs=2, space="PSUM"))

        # ---- prolog: V_ext chunks (host-built: per-head 64 v cols + ones) ----
        vext = []
        for i in range(KT):
            ve = vext_pool.tile([128, H * 65], BF16, name=f"vext{i}", tag="vext")
            nc.sync.dma_start(ve[:], vd[128 * i : 128 * (i + 1), :])
            vext.append(ve)

        wts = []
        for e in range(EC):
            wt = wt_pool.tile([128, E], wdt, name=f"wt{e}", tag="wtt")
            nc.sync.dma_start(wt[:], wTd[128 * e : 128 * (e + 1), :])
            wts.append(wt)

        # ones [32, 64] stationary for the K=32 recip-broadcast outer product
        ones32 = ones_pool.tile([32, 64], FP16, name="ones32", tag="ones32")
        nc.vector.memset(ones32[:], 1.0)

        # ---- attention ----
        attnT = []
        for j in range(HP):
            qtp = qt_pool.tile([128, NQ], wdt, name=f"qtp{j}", tag="qtp")
            nc.sync.dma_start(qtp[:], qT[128 * j : 128 * (j + 1), :])
            ktp = kt_pool.tile([128, S], wdt, name=f"ktp{j}", tag="ktp")
            nc.sync.dma_start(ktp[:], kTd[128 * j : 128 * (j + 1), :])
            at = at_pool.tile([128, NQ], wdt, name=f"at{j}", tag="at")
            attnT.append(at)
            for hh in range(2):
                h = 2 * j + hh
                r0, r1 = 64 * hh, 64 * hh + 64
                av = av_psum.tile([65, NQ], F32, name=f"av{h}", tag="av")
                pending = None
                for i in range(KT):
                    st = st_psum.tile([128, NQ], F32, name=f"st{h}_{i}", tag="st")
                    mm(st, ktp[r0:r1, 128 * i : 128 * (i + 1)], qtp[r0:r1, :],
                       start=True, stop=True)
                    pt = pt_pool.tile([128, NQ], BF16, name=f"pt{h}_{i}", tag="pt")
                    if dve_exp_mod and i % dve_exp_mod == 1:
                        td = dt_pool.tile([128, NQ], F32, name=f"td{h}_{i}", tag="td")
                        nc.vector._custom_dve(exp_ops[0], out=td[:], in0=st[:],
                                              s0=scale / 4096.0)
                        nc.vector._custom_dve(exp_ops[1], out=pt[:], in0=td[:])
                    else:
                        nc.scalar.activation(pt[:], st[:], EXP, scale=scale)
                    if not pipe:
                        mm(av, vext[i][:, 65 * h : 65 * h + 65], pt[:],
                           start=(i == 0), stop=(i == KT - 1))
                        continue
                    # pipelined: issue av(i-1) after scores(i) so PE never
                    # waits on exp(i)
                    if pending is not None:
                        mm(av, pending[0], pending[1],
                           start=(pending[2] == 0), stop=False)
                    pending = (vext[i][:, 65 * h : 65 * h + 65], pt, i)
                if pipe:
                    ve_ap, pt_ap, i = pending
                    mm(av, ve_ap, pt_ap, start=(i == 0), stop=True)
                rc = rc_pool.tile([32, NQ], FP16, name=f"rc{h}", tag="rc")
                nc.vector.memset(rc[:], 0.0)
                with nc.allow_low_precision(reason="fp16 softmax-recip broadcast"):
                    nc.vector.reciprocal(rc[0:1, :], av[64:65, :])
                bc = av_psum.tile([64, NQ], F32, name=f"bc{h}", tag="av")
                for c0 in range(0, NQ, 512):
                    c1 = min(c0 + 512, NQ)
                    nc.tensor.matmul(bc[:, c0:c1], ones32[:],
                                     rc[:, c0:c1], start=True, stop=True)
                rb = rb_pool.tile([64, NQ], F32, name=f"rb{h}", tag="rb")
                nc.vector.tensor_copy(rb[:], bc[:])
                nc.vector.tensor_mul(at[r0:r1, :], av[0:64, :], rb[:])

        # ---- projection ----
        for t in range(ST):
            pp = st_psum.tile([128, E], F32, name=f"pp{t}", tag="st")
            for e in range(EC):
                mm(pp, attnT[e][:, 128 * t : 128 * (t + 1)], wts[e][:],
                   start=(e == 0), stop=(e == EC - 1))
            ot = ot_pool.tile([128, E], F32, name=f"ot{t}", tag="ot")
            nc.vector.tensor_copy(ot[:], pp[:])
            nc.sync.dma_start(o[128 * t : 128 * (t + 1), :], ot[:])

    with ExitStack() as ctx:
        tc = ctx.enter_context(tile.TileContext(nc))
        if reps == 1:
            body(tc, ctx)
        else:
            with tc.For_i(0, reps, 1):
                body(tc, ctx)

    nc.compile()
    return nc


DT_MODE = "bf16"
PIPE = True
DVE_EXP_MOD = 0


def _get_program(reps=1):
    key = ("prog", reps, DT_MODE, PIPE, DVE_EXP_MOD)
    if key not in _CACHE:
        _CACHE[key] = build_attn_program(reps=reps, dt_mode=DT_MODE, pipe=PIPE,
                                         dve_exp_mod=DVE_EXP_MOD)
    return _CACHE[key]


def make_in_maps(q, k, v, w_out):
    import ml_dtypes
    qT = np.ascontiguousarray(np.transpose(q.reshape(B, 2, NQ, E), (0, 1, 3, 2)))
    kT = np.ascontiguousarray(np.transpose(k, (0, 2, 1)))
    wT = np.ascontiguousarray(w_out.T)
    if DT_MODE == "bf16":
        qT = qT.astype(ml_dtypes.bfloat16)
        kT = kT.astype(ml_dtypes.bfloat16)
        wT = wT.astype(ml_dtypes.bfloat16)
    vext = np.empty((B, S, H, 65), dtype=ml_dtypes.bfloat16)
    vext[:, :, :, 0:64] = v.reshape(B, S, H, 64)
    vext[:, :, :, 64] = 1.0
    vext = vext.reshape(B, S, H * 65)
    in_maps = []
    for c in range(N_CORES):
        b, half = divmod(c, 2)
        in_maps.append({"qT": qT[b, half], "kT": kT[b], "vext": vext[b], "wT": wT})
    return in_maps


def kernel(q, k, v, w_out, b_out, _reps=1):
    q = np.asarray(q, np.float32)
    k = np.asarray(k, np.float32)
    v = np.asarray(v, np.float32)
    w_out = np.asarray(w_out, np.float32)
    b_out = np.asarray(b_out, np.float32)

    nc = _get_program(_reps)
    res = run_bass_kernel_spmd(nc, make_in_maps(q, k, v, w_out),
                               core_ids=list(range(N_CORES)))
    out = np.empty((B, S, E), np.float32)
    for c in range(N_CORES):
        b, half = divmod(c, 2)
        out[b, half * NQ : (half + 1) * NQ] = res.results[c]["o"]
    out += b_out[None, None, :]
    return out
